# revision 22
# baseline (speedup 1.0000x reference)
"""GAT attention head (single head) distributed across 8 TRN2 NeuronCores.

Math (reference):
    sf   = seq @ W1                        # [N, O]
    f1   = sf @ a1 + b1                    # [N, 1]
    f2   = sf @ a2 + b2                    # [N, 1]
    lg   = f1 + f2.T                       # [N, N]
    co   = softmax(leaky_relu(lg, 0.2) + bias, axis=-1)
    out  = elu(co @ sf)                    # [N, O]

Algebra used on device (transposed layout: n on partitions, own rows r on
the free dim; softmax over n is the partition dim, reduced by the ones row
of the augmented vals matmul):

    leaky_relu(S, 0.2) = 0.2*S + 0.8*relu(S), and softmax over n is
    invariant to per-row g(r) terms, so the 0.2*f1[r] term is dropped.
    With A = 0.8*(f1[r] + f2raw[n] + b1 + b2), B = 0.2*(f2raw[n] + b2):
        logits_eff[n, r] = relu(A) + B + bias[n, r]
                         = max(A + B, B) + bias[n, r]
    which is ONE fused DVE op per chunk
        g'[n, r] = (f1b'[r] + s0[n]) max s1[n]
    with f1b' = 0.8*(f1raw + b1 + b2) broadcast, s0 = f2raw + 0.2*b2,
    s1 = 0.2*f2raw + 0.2*b2, followed by a bias add done BY THE DMA
    ENGINE (SWDGE accum_op=add) while streaming the bias chunk from HBM,
    then one batched exp on the scalar engine (no bias operand).

    f2raw comes for free as a 65th output column of the seq_fts chunk
    matmuls (augmented weights [W1 | W1 @ a2]).

    vals^T = [sf | 1s]^T @ e  -> rows 0..63 unnormalized vals, row 64 the
    softmax denominators; epilogue divides and applies elu.

All large tensors are shipped as bf16 (tolerance is 2e-2; measured error
~5e-3): per-core HBM traffic is 16MB bias + 4MB seq^T + ~1MB misc.
"""

import sys

sys.path.insert(0, "/opt/trn_rl_repo")

import numpy as np
import jax.numpy as jnp

import concourse.bacc as bacc
import concourse.bass as bass
import concourse.mybir as mybir
import concourse.tile as tile
from concourse.bass_utils import run_bass_kernel_spmd

F32 = mybir.dt.float32
F32R = mybir.dt.float32r
BF16 = mybir.dt.bfloat16
ADD = mybir.AluOpType.add
MAX = mybir.AluOpType.max
MIN = mybir.AluOpType.min
MULT = mybir.AluOpType.mult
EXP = mybir.ActivationFunctionType.Exp
COPY = mybir.ActivationFunctionType.Copy

M = 8          # cores
N = 8192       # nodes (columns of the attention matrix)
R = N // M     # rows per core (1024)
F_IN = 256
O = 64
P = 128        # partitions
SUP = 8        # chunks per super-chunk (x/e tile = [P, SUP*R])
GRP = 7        # chunks per sfaug psum group (7*65*4B < 2KB PSUM bank)

_CACHED = {}


def build_nc(n=N, r=R, core_id_hint=0):
    nch = n // P               # 128-node chunks
    nsup = nch // SUP          # super-chunks
    hs = [slice(i * 512, min((i + 1) * 512, r)) for i in range((r + 511) // 512)]
    ngrp = (nch + GRP - 1) // GRP
    gsizes = [min(GRP, nch - g * GRP) for g in range(ngrp)]

    nc = bacc.Bacc(
        "TRN2",
        target_bir_lowering=False,
        debug=False,
        enable_asserts=True,
        num_devices=M,
    )

    seqTb_d = nc.dram_tensor("seqTb", [F_IN, n], BF16, kind="ExternalInput")
    biasI_d = nc.dram_tensor("biasI", [nsup, P, SUP * r], BF16, kind="ExternalInput")
    w1_d = nc.dram_tensor("w1bf", [F_IN, O], BF16, kind="ExternalInput")
    w1t_d = nc.dram_tensor("w1tbf", [O, F_IN], BF16, kind="ExternalInput")
    a1_d = nc.dram_tensor("a1bf", [O, 1], BF16, kind="ExternalInput")
    a2_d = nc.dram_tensor("a2bf", [O, 1], BF16, kind="ExternalInput")
    b1_d = nc.dram_tensor("b1f", [1, 1], F32, kind="ExternalInput")
    b2_d = nc.dram_tensor("b2f", [P, 1], F32, kind="ExternalInput")
    onesp_d = nc.dram_tensor("onespb", [1, P], BF16, kind="ExternalInput")
    oneso_d = nc.dram_tensor("onesof", [1, O], F32R, kind="ExternalInput")
    out_d = nc.dram_tensor("out", [O, r], F32, kind="ExternalOutput")

    # own shard of seqT is shipped as its own small tensor so f1 is ready early
    seqTo_d = nc.dram_tensor("seqTo", [F_IN, r], BF16, kind="ExternalInput")

    with tile.TileContext(nc) as tc:
        with (
            tc.tile_pool(name="const", bufs=1) as cp,
            tc.tile_pool(name="x", bufs=3) as xp,
            tc.tile_pool(name="e", bufs=3) as ep,
            tc.tile_pool(name="gt", bufs=3) as gtp,
            tc.tile_pool(name="sfg", bufs=2, space="PSUM") as sfgp,
            tc.tile_pool(name="vp", bufs=1, space="PSUM") as vp,
            tc.tile_pool(name="sp", bufs=1, space="PSUM") as sp,
        ):
            # ---- constants / inputs ----
            w1a = cp.tile([P, O + 1], BF16)   # [W1 half | W1@a2 half]
            nc.scalar.dma_start(w1a[:, 0:O], w1_d.ap()[0:P, :])
            w1b = cp.tile([P, O + 1], BF16)
            nc.scalar.dma_start(w1b[:, 0:O], w1_d.ap()[P:F_IN, :])
            w1t = cp.tile([O, F_IN], BF16)
            nc.scalar.dma_start(w1t[:], w1t_d.ap())
            a1c = cp.tile([O, 1], BF16)
            nc.scalar.dma_start(a1c[:], a1_d.ap())
            a2c = cp.tile([O, 1], BF16)
            nc.scalar.dma_start(a2c[:], a2_d.ap())
            b1s = cp.tile([1, 1], F32)
            nc.scalar.dma_start(b1s[:], b1_d.ap())
            b2s = cp.tile([P, 1], F32)
            nc.scalar.dma_start(b2s[:], b2_d.ap())
            onesp = cp.tile([1, P], BF16)
            nc.scalar.dma_start(onesp[:], onesp_d.ap())
            oneso = cp.tile([1, O], F32R)
            nc.scalar.dma_start(oneso[:], oneso_d.ap())
            # own-shard seqT (small, lands first) then the full seqT
            sqao = cp.tile([P, r], BF16)
            nc.scalar.dma_start(sqao[:], seqTo_d.ap()[0:P, :])
            sqbo = cp.tile([P, r], BF16)
            nc.scalar.dma_start(sqbo[:], seqTo_d.ap()[P:F_IN, :])
            # full seqT in interleaved pieces, each its own SBUF tile so DMA
            # aggregation cannot re-merge them: early sfaug groups (and with
            # them the bias stream) start before the whole 4MB lands
            npc = min(2048, n)
            cpp = npc // P  # chunks per piece
            sqa_p, sqb_p = [], []
            for q0 in range(0, n, npc):
                ta = cp.tile([P, npc], BF16)
                nc.scalar.dma_start(ta[:], seqTb_d.ap()[0:P, q0 : q0 + npc])
                tb = cp.tile([P, npc], BF16)
                nc.scalar.dma_start(tb[:], seqTb_d.ap()[P:F_IN, q0 : q0 + npc])
                sqa_p.append(ta)
                sqb_p.append(tb)

            # ---- scalar prep: b202 = 0.2*b2, b12 = b1 + b2 ----
            b202 = cp.tile([P, 1], F32)
            nc.vector.tensor_scalar(b202[:], b2s[:], 0.2, None, op0=MULT)
            b12 = cp.tile([1, 1], F32)
            nc.vector.tensor_tensor(b12[:], b1s[:], b2s[0:1, :], ADD)

            # ---- wa2 = W1 @ a2 (param folding), into col O of w1a/w1b ----
            wa2_ps = sp.tile([P, 2], F32, tag="scratch")
            nc.tensor.matmul(wa2_ps[:, 0:1], w1t[:, 0:P], a2c[:], start=True, stop=True)
            nc.tensor.matmul(wa2_ps[:, 1:2], w1t[:, P:F_IN], a2c[:], start=True, stop=True)
            nc.vector.tensor_copy(w1a[:, O : O + 1], wa2_ps[:, 0:1])
            nc.vector.tensor_copy(w1b[:, O : O + 1], wa2_ps[:, 1:2])

            # ---- f1b' = 0.8*(f1raw + b1 + b2) broadcast to [P, r] bf16 ----
            sft_ps = sp.tile([O, r], F32, tag="scratch")
            for sl in hs:
                nc.tensor.matmul(sft_ps[:, sl], w1a[:, 0:O], sqao[:, sl], start=True, stop=False)
            for sl in hs:
                nc.tensor.matmul(sft_ps[:, sl], w1b[:, 0:O], sqbo[:, sl], start=False, stop=True)
            sft = cp.tile([O, r], BF16)
            nc.vector.tensor_copy(sft[:], sft_ps[:])
            f1_ps = sp.tile([1, r], F32, tag="scratch")
            for sl in hs:
                nc.tensor.matmul(f1_ps[:, sl], a1c[:], sft[:, sl], start=True, stop=True)
            f1row = cp.tile([1, r], BF16)
            nc.vector.tensor_scalar(f1row[:], f1_ps[:], b12[:], 0.8, op0=ADD, op1=MULT)
            f1b_ps = sp.tile([P, r], F32, tag="scratch")
            for sl in hs:
                nc.tensor.matmul(f1b_ps[:, sl], onesp[:], f1row[:, sl], start=True, stop=True)
            f1b = cp.tile([P, r], BF16)
            nc.vector.tensor_copy(f1b[:], f1b_ps[:])

            # ---- sfaug: per chunk [sf | f2raw->1.0] in [P, nch*(O+1)] bf16 ----
            sfaug = cp.tile([P, nch * (O + 1)], BF16)
            sfa3 = sfaug[:].rearrange("p (c o) -> p c o", o=O + 1)
            f2raw = cp.tile([P, nch], F32)
            s0f = cp.tile([P, nch], F32)
            s1f = cp.tile([P, nch], F32)

            def emit_group(g):
                c0 = g * GRP
                sz = gsizes[g]
                gp = sfgp.tile([P, 512], F32, tag="sfg")
                for j in range(sz):
                    c = c0 + j
                    pi, pc = c // cpp, c % cpp
                    nc.tensor.matmul(
                        gp[:, j * (O + 1) : (j + 1) * (O + 1)],
                        sqa_p[pi][:, pc * P : (pc + 1) * P],
                        w1a[:],
                        start=True,
                        stop=False,
                    )
                    nc.tensor.matmul(
                        gp[:, j * (O + 1) : (j + 1) * (O + 1)],
                        sqb_p[pi][:, pc * P : (pc + 1) * P],
                        w1b[:],
                        start=False,
                        stop=True,
                    )
                # PSUM -> sfaug (bf16)
                nc.vector.tensor_copy(
                    sfaug[:, c0 * (O + 1) : (c0 + sz) * (O + 1)],
                    gp[:, 0 : sz * (O + 1)],
                )
                # f2raw gather from col O, then per-partition scalars
                nc.vector.tensor_copy(f2raw[:, c0 : c0 + sz], sfa3[:, c0 : c0 + sz, O])
                nc.vector.tensor_scalar(
                    s0f[:, c0 : c0 + sz], f2raw[:, c0 : c0 + sz], b202[:], None, op0=ADD
                )
                nc.vector.tensor_scalar(
                    s1f[:, c0 : c0 + sz], f2raw[:, c0 : c0 + sz], 0.2, b202[:],
                    op0=MULT, op1=ADD,
                )
                # overwrite f2 col with the denominator ones
                nc.vector.memset(sfa3[:, c0 : c0 + sz, O : O + 1], 1.0)

            emit_group(0)
            if ngrp > 1:
                emit_group(1)

            # ---- main loop over super-chunks ----
            # The first PLAIN supers load bias with plain HWDGE DMAs (no g'
            # dependency, so the stream starts at t=0 on the idle scalar
            # queue) and add g' with DVE afterwards; later supers get the
            # bias added by the SWDGE DMA engine (accum_op) for free.
            PLAIN = min(4, max(nsup - 2, 0)) if nsup > 1 else 0
            vals = vp.tile([O + 1, r], F32)
            for s in range(nsup):
                x = xp.tile([P, SUP * r], BF16)
                if s < PLAIN:
                    nc.sync.dma_start(x[:], biasI_d.ap()[s])
                g = s + 2
                if g < ngrp:
                    emit_group(g)
                for k in range(SUP):
                    c = s * SUP + k
                    if s < PLAIN:
                        gt = gtp.tile([P, r], BF16)
                        nc.vector.tensor_scalar(
                            gt[:], f1b[:],
                            s0f[:, c : c + 1], s1f[:, c : c + 1],
                            op0=ADD, op1=MAX,
                        )
                        nc.vector.tensor_tensor(
                            x[:, k * r : (k + 1) * r],
                            x[:, k * r : (k + 1) * r], gt[:], ADD,
                        )
                    else:
                        nc.vector.tensor_scalar(
                            x[:, k * r : (k + 1) * r], f1b[:],
                            s0f[:, c : c + 1], s1f[:, c : c + 1],
                            op0=ADD, op1=MAX,
                        )
                if s >= PLAIN:
                    # CCE accumulate caps at 2048 elems per descriptor, so
                    # slice into <=2048-col pieces (4KB bf16/partition).
                    cw = min(2048, SUP * r)
                    for q in range(0, SUP * r, cw):
                        nc.gpsimd.dma_start(
                            x[:, q : q + cw], biasI_d.ap()[s][:, q : q + cw],
                            accum_op=ADD,
                        )
                e = ep.tile([P, SUP * r], BF16)
                nexp = 4 if s == nsup - 1 else 2
                eh = max(SUP * r // nexp, r)
                for q in range(0, SUP * r, eh):
                    nc.scalar.activation(e[:, q : q + eh], x[:, q : q + eh], EXP)
                    for k in range(q // r, min((q + eh) // r, SUP)):
                        c = s * SUP + k
                        for sl in hs:
                            nc.tensor.matmul(
                                vals[:, sl],
                                sfaug[:, c * (O + 1) : (c + 1) * (O + 1)],
                                e[:, k * r + sl.start : k * r + sl.stop],
                                start=(c == 0),
                                stop=(c == nch - 1),
                            )

            # ---- epilogue: divide by row sums, elu, out ----
            # reciprocal_approx_fast misbehaves on base_partition != 0, so
            # first move the denominator row down to partition 0.
            den_sb = cp.tile([1, r], F32)
            nc.vector.tensor_copy(den_sb[:], vals[O : O + 1, :])
            recip = cp.tile([1, r], F32)
            nc.vector.reciprocal_approx_fast(recip[:], den_sb[:])
            recipr = cp.tile([1, r], F32R)
            with nc.allow_low_precision(reason="broadcast matmul operand"):
                nc.vector.tensor_copy(recipr[:], recip[:])
            rb_ps = sp.tile([O, r], F32, tag="scratch")
            for sl in hs:
                nc.tensor.matmul(
                    rb_ps[:, sl], oneso[:], recipr[:, sl],
                    start=True, stop=True,
                )
            vals_sb = cp.tile([O, r], F32)
            nc.scalar.activation(vals_sb[:], vals[0:O, :], COPY)
            vn = cp.tile([O, r], F32)
            nc.vector.tensor_tensor(vn[:], vals_sb[:], rb_ps[:], MULT)
            # elu(x) = (relu(x) - 1) + min(exp(x), 1)   [x small, no overflow]
            p2 = cp.tile([O, r], F32)
            nc.vector.tensor_scalar(p2[:], vn[:], 0.0, -1.0, op0=MAX, op1=ADD)
            em = cp.tile([O, r], F32)
            nc.scalar.activation(em[:], vn[:], EXP)
            outT = cp.tile([O, r], F32)
            nc.vector.scalar_tensor_tensor(outT[:], em[:], 1.0, p2[:], op0=MIN, op1=ADD)
            nc.scalar.dma_start(out_d.ap(), outT[:])

    nc.compile()
    return nc


def get_nc():
    if "nc" not in _CACHED:
        _CACHED["nc"] = build_nc()
    return _CACHED["nc"]


def _bf16(a):
    return np.asarray(jnp.asarray(np.asarray(a, np.float32), jnp.bfloat16))


def make_in_maps(seq, bias_mat, W1, a1, b1, a2, b2, n=N, r=R):
    m = n // r
    nch = n // P
    nsup = nch // SUP
    seq2 = np.asarray(seq, dtype=np.float32).reshape(n, F_IN)
    bias2 = np.asarray(bias_mat, dtype=np.float32).reshape(n, n)
    seqTb = _bf16(seq2.T)
    W1f = np.asarray(W1, np.float32).reshape(F_IN, O)
    common = {
        "seqTb": seqTb,
        "w1bf": _bf16(W1f),
        "w1tbf": _bf16(W1f.T),
        "a1bf": _bf16(np.asarray(a1, np.float32).reshape(O, 1)),
        "a2bf": _bf16(np.asarray(a2, np.float32).reshape(O, 1)),
        "b1f": np.asarray(b1, np.float32).reshape(1, 1),
        "b2f": np.full((P, 1), np.float32(np.asarray(b2).reshape(())), np.float32),
        "onespb": _bf16(np.ones((1, P), np.float32)),
        "onesof": np.ones((1, O), np.float32),
    }
    in_maps = []
    for i in range(m):
        rows = slice(i * r, (i + 1) * r)
        # bias rows for this core, transposed to [n, r], then interleaved to
        # [nsup, P, SUP*r] so each super-chunk DMA is contiguous per partition:
        # biasI[s, p, k*r + j] = biasT[s*SUP*P + k*P + p, j]
        bT = bias2[rows, :].T.reshape(nsup, SUP, P, r)
        bI = _bf16(np.ascontiguousarray(bT.transpose(0, 2, 1, 3)).reshape(nsup, P, SUP * r))
        in_maps.append(
            dict(
                common,
                seqTo=_bf16(np.ascontiguousarray(seq2[rows, :].T)),
                biasI=bI,
            )
        )
    return in_maps


def kernel(seq, bias_mat, W1, a1, b1, a2, b2):
    nc = get_nc()
    in_maps = make_in_maps(seq, bias_mat, W1, a1, b1, a2, b2)
    res = run_bass_kernel_spmd(nc, in_maps, core_ids=list(range(M)))
    outs = [res.results[i]["out"] for i in range(M)]
    full = np.concatenate([o.T for o in outs], axis=0)  # [N, O]
    return full.reshape(1, N, O).astype(np.float32)


if __name__ == "__main__":
    rng = np.random.default_rng(0)
    seq = rng.standard_normal((1, N, F_IN), dtype=np.float32)
    bias = np.zeros((1, N, N), np.float32)
    W1 = (rng.standard_normal((F_IN, O)) * 0.05).astype(np.float32)
    a1 = (rng.standard_normal((O, 1)) * 0.05).astype(np.float32)
    a2 = (rng.standard_normal((O, 1)) * 0.05).astype(np.float32)
    b1 = np.zeros((1,), np.float32)
    b2 = np.zeros((1,), np.float32)
    out = kernel(seq=seq, bias_mat=bias, W1=W1, a1=a1, b1=b1, a2=a2, b2=b2)
    print(out.shape, out.dtype)


# revision 24
# speedup vs baseline: 1.2093x; 1.2093x over previous
"""GAT attention head (single head) distributed across 8 TRN2 NeuronCores.

Math (reference):
    sf   = seq @ W1                        # [N, O]
    f1   = sf @ a1 + b1                    # [N, 1]
    f2   = sf @ a2 + b2                    # [N, 1]
    lg   = f1 + f2.T                       # [N, N]
    co   = softmax(leaky_relu(lg, 0.2) + bias, axis=-1)
    out  = elu(co @ sf)                    # [N, O]

Algebra used on device (transposed layout: n on partitions, own rows r on
the free dim; softmax over n is the partition dim, reduced by the ones row
of the augmented vals matmul):

    leaky_relu(S, 0.2) = 0.2*S + 0.8*relu(S), and softmax over n is
    invariant to per-row g(r) terms, so the 0.2*f1[r] term is dropped.
    With A = 0.8*(f1[r] + f2raw[n] + b1 + b2), B = 0.2*(f2raw[n] + b2):
        logits_eff[n, r] = relu(A) + B + bias[n, r]
                         = max(A + B, B) + bias[n, r]
    which is ONE fused DVE op per chunk
        g'[n, r] = (f1b'[r] + s0[n]) max s1[n]
    with f1b' = 0.8*(f1raw + b1 + b2) broadcast, s0 = f2raw + 0.2*b2,
    s1 = 0.2*f2raw + 0.2*b2, followed by a bias add done BY THE DMA
    ENGINE (SWDGE accum_op=add) while streaming the bias chunk from HBM,
    then one batched exp on the scalar engine (no bias operand).

    f2raw comes for free as a 65th output column of the seq_fts chunk
    matmuls (augmented weights [W1 | W1 @ a2]).

    vals^T = [sf | 1s]^T @ e  -> rows 0..63 unnormalized vals, row 64 the
    softmax denominators; epilogue divides and applies elu.

All large tensors are shipped as bf16 (tolerance is 2e-2; measured error
~5e-3): per-core HBM traffic is 16MB bias + 4MB seq^T + ~1MB misc.
"""

import sys

sys.path.insert(0, "/opt/trn_rl_repo")

import numpy as np
import jax.numpy as jnp

import concourse.bacc as bacc
import concourse.bass as bass
import concourse.mybir as mybir
import concourse.tile as tile
from concourse.bass_utils import run_bass_kernel_spmd

F32 = mybir.dt.float32
F32R = mybir.dt.float32r
BF16 = mybir.dt.bfloat16
ADD = mybir.AluOpType.add
MAX = mybir.AluOpType.max
MIN = mybir.AluOpType.min
MULT = mybir.AluOpType.mult
EXP = mybir.ActivationFunctionType.Exp
COPY = mybir.ActivationFunctionType.Copy

M = 8          # cores
N = 8192       # nodes (columns of the attention matrix)
R = N // M     # rows per core (1024)
F_IN = 256
O = 64
P = 128        # partitions
SUP = 8        # chunks per super-chunk (x/e tile = [P, SUP*R])
GRP = 7        # chunks per sfaug psum group (7*65*4B < 2KB PSUM bank)

_CACHED = {}


def build_nc(n=N, r=R, core_id_hint=0):
    nch = n // P               # 128-node chunks
    nsup = nch // SUP          # super-chunks
    hs = [slice(i * 512, min((i + 1) * 512, r)) for i in range((r + 511) // 512)]
    ngrp = (nch + GRP - 1) // GRP
    gsizes = [min(GRP, nch - g * GRP) for g in range(ngrp)]

    nc = bacc.Bacc(
        "TRN2",
        target_bir_lowering=False,
        debug=False,
        enable_asserts=True,
        num_devices=M,
    )

    seqTb_d = nc.dram_tensor("seqTb", [F_IN, n], BF16, kind="ExternalInput")
    biasI_d = nc.dram_tensor("biasI", [nsup, P, SUP * r], BF16, kind="ExternalInput")
    w1_d = nc.dram_tensor("w1bf", [F_IN, O], BF16, kind="ExternalInput")
    w1t_d = nc.dram_tensor("w1tbf", [O, F_IN], BF16, kind="ExternalInput")
    a1_d = nc.dram_tensor("a1bf", [O, 1], BF16, kind="ExternalInput")
    a2_d = nc.dram_tensor("a2bf", [O, 1], BF16, kind="ExternalInput")
    b1_d = nc.dram_tensor("b1f", [1, 1], F32, kind="ExternalInput")
    b2_d = nc.dram_tensor("b2f", [P, 1], F32, kind="ExternalInput")
    onesp_d = nc.dram_tensor("onespb", [1, P], BF16, kind="ExternalInput")
    oneso_d = nc.dram_tensor("onesof", [1, O], F32R, kind="ExternalInput")
    out_d = nc.dram_tensor("out", [O, r], F32, kind="ExternalOutput")

    # own shard of seqT is shipped as its own small tensor so f1 is ready early
    seqTo_d = nc.dram_tensor("seqTo", [F_IN, r], BF16, kind="ExternalInput")

    with tile.TileContext(nc) as tc:
        with (
            tc.tile_pool(name="const", bufs=1) as cp,
            tc.tile_pool(name="x", bufs=3) as xp,
            tc.tile_pool(name="e", bufs=2) as ep,
            tc.tile_pool(name="gt", bufs=3) as gtp,
            tc.tile_pool(name="sfg", bufs=2, space="PSUM") as sfgp,
            tc.tile_pool(name="vp", bufs=1, space="PSUM") as vp,
            tc.tile_pool(name="sp", bufs=1, space="PSUM") as sp,
        ):
            # ---- constants / inputs ----
            w1a = cp.tile([P, O + 1], BF16)   # [W1 half | W1@a2 half]
            nc.scalar.dma_start(w1a[:, 0:O], w1_d.ap()[0:P, :])
            w1b = cp.tile([P, O + 1], BF16)
            nc.scalar.dma_start(w1b[:, 0:O], w1_d.ap()[P:F_IN, :])
            w1t = cp.tile([O, F_IN], BF16)
            nc.scalar.dma_start(w1t[:], w1t_d.ap())
            a1c = cp.tile([O, 1], BF16)
            nc.scalar.dma_start(a1c[:], a1_d.ap())
            a2c = cp.tile([O, 1], BF16)
            nc.scalar.dma_start(a2c[:], a2_d.ap())
            b1s = cp.tile([1, 1], F32)
            nc.scalar.dma_start(b1s[:], b1_d.ap())
            b2s = cp.tile([P, 1], F32)
            nc.scalar.dma_start(b2s[:], b2_d.ap())
            onesp = cp.tile([1, P], BF16)
            nc.scalar.dma_start(onesp[:], onesp_d.ap())
            oneso = cp.tile([1, O], F32R)
            nc.scalar.dma_start(oneso[:], oneso_d.ap())
            # own-shard seqT (small, lands first) then the full seqT
            sqao = cp.tile([P, r], BF16)
            nc.sync.dma_start(sqao[:], seqTo_d.ap()[0:P, :])
            sqbo = cp.tile([P, r], BF16)
            nc.sync.dma_start(sqbo[:], seqTo_d.ap()[P:F_IN, :])
            # full seqT in interleaved pieces, each its own SBUF tile so DMA
            # aggregation cannot re-merge them: early sfaug groups (and with
            # them the bias stream) start before the whole 4MB lands
            npc = min(2048, n)
            cpp = npc // P  # chunks per piece
            nsup_l = nch // SUP
            PLAIN = min(4, max(nsup_l - 2, 0)) if nsup_l > 1 else 0
            plain_x = {}

            def load_plain(s):
                xt = xp.tile([P, SUP * r], BF16)
                nc.sync.dma_start(xt[:], biasI_d.ap()[s])
                plain_x[s] = xt

            npieces = (n + npc - 1) // npc
            # plain supers to load right after piece i lands (hand-scheduled
            # so each consumer on the serial sync queue is fed just in time)
            sched = {0: [0], 1: [1], 2: [2, 3]} if (npieces == 4 and PLAIN == 4)                 else {npieces - 1: list(range(PLAIN))}
            sqa_p, sqb_p = [], []
            for i, q0 in enumerate(range(0, n, npc)):
                ta = cp.tile([P, npc], BF16)
                nc.sync.dma_start(ta[:], seqTb_d.ap()[0:P, q0 : q0 + npc])
                tb = cp.tile([P, npc], BF16)
                nc.sync.dma_start(tb[:], seqTb_d.ap()[P:F_IN, q0 : q0 + npc])
                sqa_p.append(ta)
                sqb_p.append(tb)
                for s in sched.get(i, []):
                    load_plain(s)

            # ---- scalar prep: b202 = 0.2*b2, b12 = b1 + b2 ----
            b202 = cp.tile([P, 1], F32)
            nc.vector.tensor_scalar(b202[:], b2s[:], 0.2, None, op0=MULT)
            b12 = cp.tile([1, 1], F32)
            nc.vector.tensor_tensor(b12[:], b1s[:], b2s[0:1, :], ADD)

            # ---- wa2 = W1 @ a2 (param folding), into col O of w1a/w1b ----
            wa2_ps = sp.tile([P, 2], F32, tag="scratch")
            nc.tensor.matmul(wa2_ps[:, 0:1], w1t[:, 0:P], a2c[:], start=True, stop=True)
            nc.tensor.matmul(wa2_ps[:, 1:2], w1t[:, P:F_IN], a2c[:], start=True, stop=True)
            nc.vector.tensor_copy(w1a[:, O : O + 1], wa2_ps[:, 0:1])
            nc.vector.tensor_copy(w1b[:, O : O + 1], wa2_ps[:, 1:2])

            # ---- f1b' = 0.8*(f1raw + b1 + b2) broadcast to [P, r] bf16 ----
            sft_ps = sp.tile([O, r], F32, tag="scratch")
            for sl in hs:
                nc.tensor.matmul(sft_ps[:, sl], w1a[:, 0:O], sqao[:, sl], start=True, stop=False)
            for sl in hs:
                nc.tensor.matmul(sft_ps[:, sl], w1b[:, 0:O], sqbo[:, sl], start=False, stop=True)
            sft = cp.tile([O, r], BF16)
            nc.vector.tensor_copy(sft[:], sft_ps[:])
            f1_ps = sp.tile([1, r], F32, tag="scratch")
            for sl in hs:
                nc.tensor.matmul(f1_ps[:, sl], a1c[:], sft[:, sl], start=True, stop=True)
            f1row = cp.tile([1, r], BF16)
            nc.vector.tensor_scalar(f1row[:], f1_ps[:], b12[:], 0.8, op0=ADD, op1=MULT)
            f1b_ps = sp.tile([P, r], F32, tag="scratch")
            for sl in hs:
                nc.tensor.matmul(f1b_ps[:, sl], onesp[:], f1row[:, sl], start=True, stop=True)
            f1b = cp.tile([P, r], BF16)
            nc.vector.tensor_copy(f1b[:], f1b_ps[:])

            # ---- sfaug: per chunk [sf | f2raw->1.0] in [P, nch*(O+1)] bf16 ----
            sfaug = cp.tile([P, nch * (O + 1)], BF16)
            sfa3 = sfaug[:].rearrange("p (c o) -> p c o", o=O + 1)
            f2raw = cp.tile([P, nch], F32)
            s0f = cp.tile([P, nch], F32)
            s1f = cp.tile([P, nch], F32)

            def emit_group(g):
                c0 = g * GRP
                sz = gsizes[g]
                gp = sfgp.tile([P, 512], F32, tag="sfg")
                for j in range(sz):
                    c = c0 + j
                    pi, pc = c // cpp, c % cpp
                    nc.tensor.matmul(
                        gp[:, j * (O + 1) : (j + 1) * (O + 1)],
                        sqa_p[pi][:, pc * P : (pc + 1) * P],
                        w1a[:],
                        start=True,
                        stop=False,
                    )
                    nc.tensor.matmul(
                        gp[:, j * (O + 1) : (j + 1) * (O + 1)],
                        sqb_p[pi][:, pc * P : (pc + 1) * P],
                        w1b[:],
                        start=False,
                        stop=True,
                    )
                # PSUM -> sfaug (bf16)
                nc.vector.tensor_copy(
                    sfaug[:, c0 * (O + 1) : (c0 + sz) * (O + 1)],
                    gp[:, 0 : sz * (O + 1)],
                )
                # f2raw gather from col O, then per-partition scalars
                nc.vector.tensor_copy(f2raw[:, c0 : c0 + sz], sfa3[:, c0 : c0 + sz, O])
                nc.vector.tensor_scalar(
                    s0f[:, c0 : c0 + sz], f2raw[:, c0 : c0 + sz], b202[:], None, op0=ADD
                )
                nc.vector.tensor_scalar(
                    s1f[:, c0 : c0 + sz], f2raw[:, c0 : c0 + sz], 0.2, b202[:],
                    op0=MULT, op1=ADD,
                )
                # overwrite f2 col with the denominator ones
                nc.vector.memset(sfa3[:, c0 : c0 + sz, O : O + 1], 1.0)

            emit_group(0)
            if ngrp > 1:
                emit_group(1)

            # ---- main loop over super-chunks ----
            # The first PLAIN supers load bias with plain HWDGE DMAs (no g'
            # dependency, so the stream starts at t=0 on the idle scalar
            # queue) and add g' with DVE afterwards; later supers get the
            # bias added by the SWDGE DMA engine (accum_op) for free.
            vals = vp.tile([O + 1, r], F32)
            for s in range(nsup):
                x = plain_x[s] if s < PLAIN else xp.tile([P, SUP * r], BF16)
                g = s + 2
                if g < ngrp:
                    emit_group(g)
                for k in range(SUP):
                    c = s * SUP + k
                    if s < PLAIN:
                        gt = gtp.tile([P, r], BF16)
                        nc.vector.tensor_scalar(
                            gt[:], f1b[:],
                            s0f[:, c : c + 1], s1f[:, c : c + 1],
                            op0=ADD, op1=MAX,
                        )
                        nc.vector.tensor_tensor(
                            x[:, k * r : (k + 1) * r],
                            x[:, k * r : (k + 1) * r], gt[:], ADD,
                        )
                    else:
                        nc.vector.tensor_scalar(
                            x[:, k * r : (k + 1) * r], f1b[:],
                            s0f[:, c : c + 1], s1f[:, c : c + 1],
                            op0=ADD, op1=MAX,
                        )
                if s >= PLAIN:
                    # CCE accumulate caps at 2048 elems per descriptor, so
                    # slice into <=2048-col pieces (4KB bf16/partition).
                    cw = min(2048, SUP * r)
                    for q in range(0, SUP * r, cw):
                        nc.gpsimd.dma_start(
                            x[:, q : q + cw], biasI_d.ap()[s][:, q : q + cw],
                            accum_op=ADD,
                        )
                e = ep.tile([P, SUP * r], BF16)
                nexp = 4 if s == nsup - 1 else 2
                eh = max(SUP * r // nexp, r)
                for q in range(0, SUP * r, eh):
                    nc.scalar.activation(e[:, q : q + eh], x[:, q : q + eh], EXP)
                    for k in range(q // r, min((q + eh) // r, SUP)):
                        c = s * SUP + k
                        for sl in hs:
                            nc.tensor.matmul(
                                vals[:, sl],
                                sfaug[:, c * (O + 1) : (c + 1) * (O + 1)],
                                e[:, k * r + sl.start : k * r + sl.stop],
                                start=(c == 0),
                                stop=(c == nch - 1),
                            )

            # ---- epilogue: divide by row sums, elu, out ----
            # reciprocal_approx_fast misbehaves on base_partition != 0, so
            # first move the denominator row down to partition 0.
            den_sb = cp.tile([1, r], F32)
            nc.vector.tensor_copy(den_sb[:], vals[O : O + 1, :])
            recip = cp.tile([1, r], F32)
            nc.vector.reciprocal_approx_fast(recip[:], den_sb[:])
            recipr = cp.tile([1, r], F32R)
            with nc.allow_low_precision(reason="broadcast matmul operand"):
                nc.vector.tensor_copy(recipr[:], recip[:])
            rb_ps = sp.tile([O, r], F32, tag="scratch")
            for sl in hs:
                nc.tensor.matmul(
                    rb_ps[:, sl], oneso[:], recipr[:, sl],
                    start=True, stop=True,
                )
            vals_sb = cp.tile([O, r], F32)
            nc.scalar.activation(vals_sb[:], vals[0:O, :], COPY)
            vn = cp.tile([O, r], F32)
            nc.vector.tensor_tensor(vn[:], vals_sb[:], rb_ps[:], MULT)
            # elu(x) = (relu(x) - 1) + min(exp(x), 1)   [x small, no overflow]
            p2 = cp.tile([O, r], F32)
            nc.vector.tensor_scalar(p2[:], vn[:], 0.0, -1.0, op0=MAX, op1=ADD)
            em = cp.tile([O, r], F32)
            nc.scalar.activation(em[:], vn[:], EXP)
            outT = cp.tile([O, r], F32)
            nc.vector.scalar_tensor_tensor(outT[:], em[:], 1.0, p2[:], op0=MIN, op1=ADD)
            nc.scalar.dma_start(out_d.ap(), outT[:])

    nc.compile()
    return nc


def get_nc():
    if "nc" not in _CACHED:
        _CACHED["nc"] = build_nc()
    return _CACHED["nc"]


def _bf16(a):
    return np.asarray(jnp.asarray(np.asarray(a, np.float32), jnp.bfloat16))


def make_in_maps(seq, bias_mat, W1, a1, b1, a2, b2, n=N, r=R):
    m = n // r
    nch = n // P
    nsup = nch // SUP
    seq2 = np.asarray(seq, dtype=np.float32).reshape(n, F_IN)
    bias2 = np.asarray(bias_mat, dtype=np.float32).reshape(n, n)
    seqTb = _bf16(seq2.T)
    W1f = np.asarray(W1, np.float32).reshape(F_IN, O)
    common = {
        "seqTb": seqTb,
        "w1bf": _bf16(W1f),
        "w1tbf": _bf16(W1f.T),
        "a1bf": _bf16(np.asarray(a1, np.float32).reshape(O, 1)),
        "a2bf": _bf16(np.asarray(a2, np.float32).reshape(O, 1)),
        "b1f": np.asarray(b1, np.float32).reshape(1, 1),
        "b2f": np.full((P, 1), np.float32(np.asarray(b2).reshape(())), np.float32),
        "onespb": _bf16(np.ones((1, P), np.float32)),
        "onesof": np.ones((1, O), np.float32),
    }
    in_maps = []
    for i in range(m):
        rows = slice(i * r, (i + 1) * r)
        # bias rows for this core, transposed to [n, r], then interleaved to
        # [nsup, P, SUP*r] so each super-chunk DMA is contiguous per partition:
        # biasI[s, p, k*r + j] = biasT[s*SUP*P + k*P + p, j]
        bT = bias2[rows, :].T.reshape(nsup, SUP, P, r)
        bI = _bf16(np.ascontiguousarray(bT.transpose(0, 2, 1, 3)).reshape(nsup, P, SUP * r))
        in_maps.append(
            dict(
                common,
                seqTo=_bf16(np.ascontiguousarray(seq2[rows, :].T)),
                biasI=bI,
            )
        )
    return in_maps


def kernel(seq, bias_mat, W1, a1, b1, a2, b2):
    nc = get_nc()
    in_maps = make_in_maps(seq, bias_mat, W1, a1, b1, a2, b2)
    res = run_bass_kernel_spmd(nc, in_maps, core_ids=list(range(M)))
    outs = [res.results[i]["out"] for i in range(M)]
    full = np.concatenate([o.T for o in outs], axis=0)  # [N, O]
    return full.reshape(1, N, O).astype(np.float32)


if __name__ == "__main__":
    rng = np.random.default_rng(0)
    seq = rng.standard_normal((1, N, F_IN), dtype=np.float32)
    bias = np.zeros((1, N, N), np.float32)
    W1 = (rng.standard_normal((F_IN, O)) * 0.05).astype(np.float32)
    a1 = (rng.standard_normal((O, 1)) * 0.05).astype(np.float32)
    a2 = (rng.standard_normal((O, 1)) * 0.05).astype(np.float32)
    b1 = np.zeros((1,), np.float32)
    b2 = np.zeros((1,), np.float32)
    out = kernel(seq=seq, bias_mat=bias, W1=W1, a1=a1, b1=b1, a2=a2, b2=b2)
    print(out.shape, out.dtype)
